# revision 3
# baseline (speedup 1.0000x reference)
"""DLinear (causal-window decomposition + dual Linear) as a single fused matmul
on 8 Trainium2 NeuronCores.

Algebra: with A the [T,T] causal-window-mean operator (banded, window=25),
    out = x @ (Sw + (Tw - Sw) @ A)^T + (tb + sb)
A is banded, so the fold happens on the host in O(T^2); the device runs one
[2048,721] x [721,720] matmul per core with the bias riding as an extra
contraction row against a ones row appended to x^T.

Device schedule (v3): x-stationary, which streams the minimal column count
(16 m-tiles x 6 k-chunks x 720 w-columns = 69120 cycles ~= 28.8us at the
1 col/cycle fp16 rate; the PE hides LDWEIGHTS).  Per m-tile, the k loop
issues the n0/n1 matmuls back to back into two PSUM banks.  The lead-in is
fine-grained: per-k-chunk DMAs for W (both halves interleaved, scalar ring)
and for the first m-tile of x (sync ring), so real matmuls start ~1.5us in
and run cold until the HAM window flips -- no junk-warmup block serializing
4us ahead of real work.  Stores go per m-tile inline (the last one split in
quarters) so no output tail builds up.  fp16 end to end, fp32 PSUM.
"""

import sys
import types

import numpy as np

import concourse.bacc as bacc
import concourse.mybir as mybir
from concourse import tile
from concourse.bass_utils import run_bass_kernel_spmd

try:
    import antenv.axon_hooks  # noqa: F401
except ImportError:
    try:
        import antenv
        _shim = types.ModuleType("antenv.axon_hooks")
        _shim._hook = None
        _shim.set_axon_ntff_profile_hook = lambda h: setattr(_shim, "_hook", h)
        _shim.get_axon_ntff_profile_hook = lambda: _shim._hook
        sys.modules["antenv.axon_hooks"] = _shim
        antenv.axon_hooks = _shim
    except ImportError:
        pass

WINDOW = 25
B, NPTS, T = 32, 512, 720
U = T                     # output features
N_CORES = 8
M_TOT = B * NPTS          # 16384 rows
M_LOC = M_TOT // N_CORES  # 2048 rows per core
P = 128                   # partitions
M_TILES = M_LOC // P      # 16
KE = T + 1                # contraction incl. bias row = 721
KFULL = KE // P           # 5 full 128-row k-chunks
KREM = KE - KFULL * P     # 81 rows in the last chunk
NKC = KFULL + 1           # 6 k-chunks
N_CHUNKS = [(0, 360), (360, 360)]   # PSUM-bank-sized halves of U
# x column groups after the fine-grained first m-tile
X_GROUPS = [(128, 512), (640, 512), (1152, 512), (1664, 384)]

_F32 = mybir.dt.float32
_F16 = mybir.dt.float16
N_WARMUP = 3              # junk matmuls bridging the first DMA latency


def _kc(k):
    return P if k < KFULL else KREM


def _build_nc():
    nc = bacc.Bacc("TRN2", target_bir_lowering=False, debug=False,
                   num_devices=N_CORES, enable_partition_id=False)
    xt_d = nc.dram_tensor("xt", [KE, M_LOC], _F16, kind="ExternalInput").ap()
    wt_d = nc.dram_tensor("wt", [KE, U], _F16, kind="ExternalInput").ap()
    out_d = nc.dram_tensor("out", [M_LOC, U], _F16, kind="ExternalOutput").ap()
    xt_main = xt_d[0:KFULL * P, :].rearrange("(k p) m -> p k m", p=P)
    xt_rem = xt_d[KFULL * P:KE, :]

    with tile.TileContext(nc) as tc:
        with tc.tile_pool(name="wpool", bufs=1) as wpool, \
             tc.tile_pool(name="xpool", bufs=1) as xpool, \
             tc.tile_pool(name="opool", bufs=4) as opool, \
             tc.tile_pool(name="jp", bufs=1, space="PSUM") as jp, \
             tc.tile_pool(name="accp", bufs=6, space="PSUM") as accp:

            # Junk matmuls: start the HAM activity window and bridge the
            # first DMA completion latency.
            scr = wpool.tile([P, 640], _F16, name="scr", tag="scr")
            nc.gpsimd.memset(scr[:], 0.0)
            ps_scr = jp.tile([P, 512], _F32, name="ps_scr", tag="ps_scr")
            for _ in range(N_WARMUP):
                nc.tensor.matmul(ps_scr[:], scr[:, 0:P], scr[:, P:P + 512],
                                 start=True, stop=True)

            w_all = wpool.tile([P, NKC * U], _F16, name="w_all", tag="w_all")
            w_v = w_all[:].rearrange("p (k u) -> p k u", k=NKC)
            x_all = xpool.tile([P, NKC * M_LOC], _F16, name="x_all",
                               tag="x_all")
            x_v = x_all[:].rearrange("p (k m) -> p k m", k=NKC)

            # Scalar ring: W per (k-chunk, n-half), k-major so the first
            # m-tile's k-sweep is gated chunk by chunk, not by bulk loads.
            for k in range(NKC):
                for n0, nw in N_CHUNKS:
                    nc.scalar.dma_start(
                        w_v[0:_kc(k), k, n0:n0 + nw],
                        wt_d[k * P:k * P + _kc(k), n0:n0 + nw])
            # Sync ring: x -- per-k chunks for m-tile 0, then bulk groups.
            for k in range(NKC):
                nc.sync.dma_start(x_v[0:_kc(k), k, 0:P],
                                  xt_d[k * P:k * P + _kc(k), 0:P])
            for c0, cw in X_GROUPS:
                cs = slice(c0, c0 + cw)
                nc.sync.dma_start(x_v[:, 0:KFULL, cs], xt_main[:, :, cs])
                nc.sync.dma_start(x_v[0:KREM, KFULL, cs], xt_rem[:, cs])

            for m in range(M_TILES):
                ms = slice(m * P, (m + 1) * P)
                accs = [accp.tile([P, 512], _F32, name="acc", tag="acc")
                        for _ in N_CHUNKS]
                for k in range(NKC):
                    kc = _kc(k)
                    for n, (n0, nw) in enumerate(N_CHUNKS):
                        nc.tensor.matmul(accs[n][:, 0:nw],
                                         x_v[0:kc, k, ms],
                                         w_v[0:kc, k, n0:n0 + nw],
                                         start=(k == 0), stop=(k == NKC - 1))
                ot = opool.tile([P, U], _F16, name="ot")
                if m == M_TILES - 1:
                    # quarter the final copy+stores so the tail is one small
                    # store, not a whole m-tile
                    for n, (n0, nw) in enumerate(N_CHUNKS):
                        for h0 in (0, nw // 2):
                            hs = slice(n0 + h0, n0 + h0 + nw // 2)
                            nc.vector.tensor_copy(
                                ot[:, hs], accs[n][:, h0:h0 + nw // 2])
                            nc.scalar.dma_start(out_d[ms, hs], ot[:, hs])
                else:
                    for n, (n0, nw) in enumerate(N_CHUNKS):
                        nc.vector.tensor_copy(ot[:, n0:n0 + nw],
                                              accs[n][:, 0:nw])
                    nc.scalar.dma_start(out_d[ms, :], ot[:])

    nc.compile()
    return nc


def _fold_weights(trend_w, seasonal_w, trend_b, seasonal_b):
    """W = seasonal_w + (trend_w - seasonal_w) @ A via the banded structure of
    A; returns [KE, U] = [W^T; b] ready for the device."""
    trend_w = np.asarray(trend_w, dtype=np.float64)
    seasonal_w = np.asarray(seasonal_w, dtype=np.float64)
    trend_b = np.asarray(trend_b, dtype=np.float64)
    seasonal_b = np.asarray(seasonal_b, dtype=np.float64)
    counts = np.minimum(np.arange(T) + 1, WINDOW).astype(np.float64)
    G = (trend_w - seasonal_w) / counts[None, :]
    M = np.zeros_like(G)
    for d in range(WINDOW):
        M[:, :T - d] += G[:, d:]
    W = seasonal_w + M
    b = trend_b + seasonal_b
    wt_ext = np.empty((KE, U), np.float32)
    wt_ext[:T, :] = W.T.astype(np.float32)
    wt_ext[T, :] = b.astype(np.float32)
    return wt_ext


_NC_CACHE = {}
RUN_KWARGS = {}   # test harness may set {"trace": True}
LAST_RESULTS = None


def kernel(x, trend_w, trend_b, seasonal_w, seasonal_b):
    global LAST_RESULTS
    wt16 = _fold_weights(trend_w, seasonal_w, trend_b,
                         seasonal_b).astype(np.float16)

    # Pre-transposed, ones-row-extended fp16 shards [721, 2048] per core.
    x2d = np.asarray(x, dtype=np.float32).reshape(M_TOT, T)
    xt_all = np.empty((KE, M_TOT), np.float16)
    xt_all[:T] = x2d.T.astype(np.float16)
    xt_all[T] = 1.0
    xt_cores = [np.ascontiguousarray(xt_all[:, i * M_LOC:(i + 1) * M_LOC])
                for i in range(N_CORES)]

    if "nc" not in _NC_CACHE:
        _NC_CACHE["nc"] = _build_nc()
    nc = _NC_CACHE["nc"]

    in_maps = [{"xt": xt_cores[i], "wt": wt16} for i in range(N_CORES)]
    res = run_bass_kernel_spmd(nc, in_maps, core_ids=list(range(N_CORES)),
                               **RUN_KWARGS)
    LAST_RESULTS = res
    out = np.concatenate([r["out"] for r in res.results], axis=0)
    return out.astype(np.float32).reshape(B, NPTS, U)


# revision 4
# speedup vs baseline: 1.0797x; 1.0797x over previous
"""DLinear (causal-window decomposition + dual Linear) as a single fused matmul
on 8 Trainium2 NeuronCores.

Algebra: with A the [T,T] causal-window-mean operator (banded, window=25),
    out = x @ (Sw + (Tw - Sw) @ A)^T + (tb + sb)
A is banded, so the fold happens on the host in O(T^2); the device runs one
[2048,721] x [721,720] matmul per core with the bias riding as an extra
contraction row against a ones row appended to x^T.

Device schedule (v4): x-stationary -- minimal stream count (16 m-tiles x
6 k-chunks x 720 w-cols = 69120 PE cycles ~= 28.8us at the 1 col/cycle fp16
rate; LDWEIGHTS is hidden by the PE reorder window).  Bulk DMAs only (small
transfers crater DMA efficiency): W in two 360-column halves on the scalar
ring, x in five 721-row column groups on the sync ring, so the two rings
drain in parallel.  A junk-matmul warmup bridges the DMA ramp and flips the
HAM clock gate before real matmuls start.  Per m-tile the k loop feeds the
two 360-column PSUM banks back to back; stores go out per n-half right after
each bank's DVE copy, and the final tile's stores are quartered so the kernel
ends on a small transfer instead of a serial output tail.  fp16 end to end,
fp32 PSUM accumulation (~4e-4 rel err).
"""

import sys
import types

import numpy as np

import concourse.bacc as bacc
import concourse.mybir as mybir
from concourse import tile
from concourse.bass_utils import run_bass_kernel_spmd

try:
    import antenv.axon_hooks  # noqa: F401
except ImportError:
    try:
        import antenv
        _shim = types.ModuleType("antenv.axon_hooks")
        _shim._hook = None
        _shim.set_axon_ntff_profile_hook = lambda h: setattr(_shim, "_hook", h)
        _shim.get_axon_ntff_profile_hook = lambda: _shim._hook
        sys.modules["antenv.axon_hooks"] = _shim
        antenv.axon_hooks = _shim
    except ImportError:
        pass

WINDOW = 25
B, NPTS, T = 32, 512, 720
U = T                     # output features
N_CORES = 8
M_TOT = B * NPTS          # 16384 rows
M_LOC = M_TOT // N_CORES  # 2048 rows per core
P = 128                   # partitions
M_TILES = M_LOC // P      # 16
KE = T + 1                # contraction incl. bias row = 721
KFULL = KE // P           # 5 full 128-row k-chunks
KREM = KE - KFULL * P     # 81 rows in the last chunk
NKC = KFULL + 1           # 6 k-chunks
KP = NKC * P              # 768 row-padded extent (W only)
N_CHUNKS = [(0, 360), (360, 360)]   # PSUM-bank-sized halves of U
# x column groups; first small so the first m-tile's data lands early
X_GROUPS = [(0, 256), (256, 384), (640, 512), (1152, 512), (1664, 384)]

_F32 = mybir.dt.float32
_F16 = mybir.dt.float16
N_WARMUP = 10             # junk matmuls covering DMA ramp + HAM window


def _build_nc():
    nc = bacc.Bacc("TRN2", target_bir_lowering=False, debug=False,
                   num_devices=N_CORES, enable_partition_id=False)
    xt_d = nc.dram_tensor("xt", [KE, M_LOC], _F16, kind="ExternalInput").ap()
    wt_d = nc.dram_tensor("wt", [KP, U], _F16, kind="ExternalInput").ap()
    out_d = nc.dram_tensor("out", [M_LOC, U], _F16, kind="ExternalOutput").ap()
    xt_main = xt_d[0:KFULL * P, :].rearrange("(k p) m -> p k m", p=P)
    xt_rem = xt_d[KFULL * P:KE, :]
    wt_v = wt_d.rearrange("(k p) u -> p k u", p=P)

    with tile.TileContext(nc) as tc:
        with tc.tile_pool(name="wpool", bufs=1) as wpool, \
             tc.tile_pool(name="xpool", bufs=1) as xpool, \
             tc.tile_pool(name="opool", bufs=4) as opool, \
             tc.tile_pool(name="jp", bufs=1, space="PSUM") as jp, \
             tc.tile_pool(name="accp", bufs=6, space="PSUM") as accp:

            # Junk matmuls: keep the PE busy through the DMA ramp so the HAM
            # clock gate lifts before real matmuls begin.
            scr = wpool.tile([P, 384], _F16, name="scr", tag="scr")
            nc.gpsimd.memset(scr[:], 0.0)
            ps_scr = jp.tile([P, 384], _F32, name="ps_scr", tag="ps_scr")
            for _ in range(N_WARMUP):
                nc.tensor.matmul(ps_scr[:], scr[:, 0:P], scr[:],
                                 start=True, stop=True)

            w_all = wpool.tile([P, NKC * U], _F16, name="w_all", tag="w_all")
            w_v = w_all[:].rearrange("p (k u) -> p k u", k=NKC)
            x_all = xpool.tile([P, NKC * M_LOC], _F16, name="x_all",
                               tag="x_all")
            x_v = x_all[:].rearrange("p (k m) -> p k m", k=NKC)

            # W halves on the scalar ring (runs in parallel with x on the
            # sync ring; stores join this ring only after W has drained).
            nc.scalar.dma_start(w_v[:, :, 0:360], wt_v[:, :, 0:360])
            nc.scalar.dma_start(w_v[:, :, 360:720], wt_v[:, :, 360:720])
            # x column groups, 721 rows (no pad bytes): main 5 chunks + rest.
            for c0, cw in X_GROUPS:
                cs = slice(c0, c0 + cw)
                nc.sync.dma_start(x_v[:, 0:KFULL, cs], xt_main[:, :, cs])
                nc.sync.dma_start(x_v[0:KREM, KFULL, cs], xt_rem[:, cs])

            for m in range(M_TILES):
                ms = slice(m * P, (m + 1) * P)
                accs = [accp.tile([P, 512], _F32, name="acc", tag="acc")
                        for _ in N_CHUNKS]
                for k in range(NKC):
                    kc = P if k < KFULL else KREM
                    for n, (n0, nw) in enumerate(N_CHUNKS):
                        nc.tensor.matmul(accs[n][:, 0:nw],
                                         x_v[0:kc, k, ms],
                                         w_v[0:kc, k, n0:n0 + nw],
                                         start=(k == 0), stop=(k == NKC - 1))
                ot = opool.tile([P, U], _F16, name="ot")
                if m == M_TILES - 1:
                    # quarter the final copies+stores: the kernel's tail is
                    # one 180-column store instead of a whole m-tile
                    for n, (n0, nw) in enumerate(N_CHUNKS):
                        for h0 in (0, nw // 2):
                            hs = slice(n0 + h0, n0 + h0 + nw // 2)
                            nc.vector.tensor_copy(
                                ot[:, hs], accs[n][:, h0:h0 + nw // 2])
                            nc.scalar.dma_start(out_d[ms, hs], ot[:, hs])
                else:
                    for n, (n0, nw) in enumerate(N_CHUNKS):
                        nc.vector.tensor_copy(ot[:, n0:n0 + nw],
                                              accs[n][:, 0:nw])
                        nc.scalar.dma_start(out_d[ms, n0:n0 + nw],
                                            ot[:, n0:n0 + nw])

    nc.compile()
    return nc


def _fold_weights(trend_w, seasonal_w, trend_b, seasonal_b):
    """W = seasonal_w + (trend_w - seasonal_w) @ A via the banded structure of
    A; returns [KE, U] = [W^T; b] ready for the device."""
    trend_w = np.asarray(trend_w, dtype=np.float64)
    seasonal_w = np.asarray(seasonal_w, dtype=np.float64)
    trend_b = np.asarray(trend_b, dtype=np.float64)
    seasonal_b = np.asarray(seasonal_b, dtype=np.float64)
    counts = np.minimum(np.arange(T) + 1, WINDOW).astype(np.float64)
    G = (trend_w - seasonal_w) / counts[None, :]
    M = np.zeros_like(G)
    for d in range(WINDOW):
        M[:, :T - d] += G[:, d:]
    W = seasonal_w + M
    b = trend_b + seasonal_b
    wt_ext = np.empty((KE, U), np.float32)
    wt_ext[:T, :] = W.T.astype(np.float32)
    wt_ext[T, :] = b.astype(np.float32)
    return wt_ext


_NC_CACHE = {}
RUN_KWARGS = {}   # test harness may set {"trace": True}
LAST_RESULTS = None


def kernel(x, trend_w, trend_b, seasonal_w, seasonal_b):
    global LAST_RESULTS
    wt_ext = _fold_weights(trend_w, seasonal_w, trend_b, seasonal_b)
    wt16 = np.zeros((KP, U), np.float16)
    wt16[:KE] = wt_ext.astype(np.float16)

    # Pre-transposed, ones-row-extended fp16 shards [721, 2048] per core.
    x2d = np.asarray(x, dtype=np.float32).reshape(M_TOT, T)
    xt_all = np.empty((KE, M_TOT), np.float16)
    xt_all[:T] = x2d.T.astype(np.float16)
    xt_all[T] = 1.0
    xt_cores = [np.ascontiguousarray(xt_all[:, i * M_LOC:(i + 1) * M_LOC])
                for i in range(N_CORES)]

    if "nc" not in _NC_CACHE:
        _NC_CACHE["nc"] = _build_nc()
    nc = _NC_CACHE["nc"]

    in_maps = [{"xt": xt_cores[i], "wt": wt16} for i in range(N_CORES)]
    res = run_bass_kernel_spmd(nc, in_maps, core_ids=list(range(N_CORES)),
                               **RUN_KWARGS)
    LAST_RESULTS = res
    out = np.concatenate([r["out"] for r in res.results], axis=0)
    return out.astype(np.float32).reshape(B, NPTS, U)


# revision 5
# speedup vs baseline: 1.1722x; 1.0857x over previous
"""DLinear (causal-window decomposition + dual Linear) as a single fused matmul
on 8 Trainium2 NeuronCores.

Algebra: with A the [T,T] causal-window-mean operator (banded, window=25),
    out = x @ (Sw + (Tw - Sw) @ A)^T + (tb + sb)
A is banded, so the fold happens on the host in O(T^2); the device runs one
[2048,721] x [721,720] matmul per core, the bias riding as an extra
contraction row against a ones row appended to x^T.

Schedule (v5).  x-stationary: stationary = x k-chunk [kc<=128, 128 m], moving
= W k-chunk [kc, 360], PSUM [128, 360] accumulated over 6 k-chunks -- this
streams the minimal 16*6*720 = 69120 PE columns (~28.8us at the measured
1 col/cycle fp16 rate; LDWEIGHTS hides under the stream).  Mechanics learned
from NTFF traces: both HWDGE rings share the 16 SDMA engines (~330 GB/s
total), each dma_start costs ~0.6-2us of its issuing sequencer, stores
serialize ~1us apiece on the Act sequencer, and exec time carries ~10us of
fixed NEFF boot+teardown.  So: bulk input DMAs only, ordered so each tile
lands just before its consumer (w-n0, x[m0-1], x[m2-5], w-n1, rest); an
11-matmul junk warmup covers the first DMA wait and flips the HAM clock
gate; a 4-tile n0-sweep gives w-n1 slack, then units interleave
(m_i,n1),(m_{i+4},n0) so each m-tile finishes early and its store issues
immediately (stores start ~8.5us vs 12us before); the final tile's two
half-stores split across the sync and scalar rings so the kernel ends on a
~1us tail.  fp16 end to end, fp32 PSUM (~4.4e-4 rel err; fp8 DoubleRow would
halve the stream but measures 3.4e-2 -- over the 2e-2 gate)."""

import sys
import types

import numpy as np

import concourse.bacc as bacc
import concourse.mybir as mybir
from concourse import tile
from concourse.bass_utils import run_bass_kernel_spmd

try:
    import antenv.axon_hooks  # noqa: F401
except ImportError:
    try:
        import antenv
        _shim = types.ModuleType("antenv.axon_hooks")
        _shim._hook = None
        _shim.set_axon_ntff_profile_hook = lambda h: setattr(_shim, "_hook", h)
        _shim.get_axon_ntff_profile_hook = lambda: _shim._hook
        sys.modules["antenv.axon_hooks"] = _shim
        antenv.axon_hooks = _shim
    except ImportError:
        pass

WINDOW = 25
B, NPTS, T = 32, 512, 720
U = T                     # output features
N_CORES = 8
M_TOT = B * NPTS          # 16384 rows
M_LOC = M_TOT // N_CORES  # 2048 rows per core
P = 128                   # partitions
M_TILES = M_LOC // P      # 16
KE = T + 1                # contraction incl. bias row
K_CHUNKS = [(k * P, min(P, KE - k * P)) for k in range((KE + P - 1) // P)]
NK = len(K_CHUNKS)        # 6
KP = NK * P               # 768, row-padded contraction extent
N_CHUNKS = [(0, 360), (360, 360)]  # PSUM-bank-sized slices of U
# x column groups sized to land just ahead of the unit schedule below
X_GROUPS = [(0, 256), (256, 512), (768, 512), (1280, 512), (1792, 256)]
SWEEP = 4                 # leading n0-only units, buys slack for the w-n1 DMA

_F32 = mybir.dt.float32
_F16 = mybir.dt.float16
N_WARMUP = 11             # junk matmuls covering first-DMA wait + HAM window


def _build_nc():
    nc = bacc.Bacc("TRN2", target_bir_lowering=False, debug=False,
                   num_devices=N_CORES, enable_partition_id=False)
    xt_d = nc.dram_tensor("xt", [KP, M_LOC], _F16, kind="ExternalInput").ap()
    wt_d = nc.dram_tensor("wt", [KP, U], _F16, kind="ExternalInput").ap()
    out_d = nc.dram_tensor("out", [M_LOC, U], _F16, kind="ExternalOutput").ap()
    xt_v = xt_d.rearrange("(k p) m -> p k m", p=P)
    wt_v = wt_d.rearrange("(k p) u -> p k u", p=P)

    with tile.TileContext(nc) as tc:
        with tc.tile_pool(name="wpool", bufs=1) as wpool, \
             tc.tile_pool(name="xpool", bufs=1) as xpool, \
             tc.tile_pool(name="opool", bufs=5) as opool, \
             tc.tile_pool(name="jp", bufs=1, space="PSUM") as jp, \
             tc.tile_pool(name="accp", bufs=7, space="PSUM") as accp:

            scr = wpool.tile([P, 384], _F16, name="scr", tag="scr")
            nc.gpsimd.memset(scr[:], 0.0)
            ps_scr = jp.tile([P, 384], _F32, name="ps_scr", tag="ps_scr")
            for _ in range(N_WARMUP):
                nc.tensor.matmul(ps_scr[:], scr[:, 0:P], scr[:],
                                 start=True, stop=True)

            w_all = wpool.tile([P, NK * U], _F16, name="w_all", tag="w_all")
            w_v = w_all[:].rearrange("p (k u) -> p k u", k=NK)
            x_all = xpool.tile([P, NK * M_LOC], _F16, name="x_all",
                               tag="x_all")
            x_v = x_all[:].rearrange("p (k m) -> p k m", k=NK)

            # All inputs on the sync ring, in consumption order.
            nc.sync.dma_start(w_v[:, :, 0:360], wt_v[:, :, 0:360])
            for c0, cw in X_GROUPS[:2]:
                nc.sync.dma_start(x_v[:, :, c0:c0 + cw],
                                  xt_v[:, :, c0:c0 + cw])
            nc.sync.dma_start(w_v[:, :, 360:720], wt_v[:, :, 360:720])
            for c0, cw in X_GROUPS[2:]:
                nc.sync.dma_start(x_v[:, :, c0:c0 + cw],
                                  xt_v[:, :, c0:c0 + cw])

            # Unit plan: n0-sweep over the first SWEEP m-tiles, then
            # (m_i, n1), (m_{i+SWEEP}, n0) pairs -- every m-tile completes
            # right after its n1 unit, so stores flow from ~8.5us on.
            plan = [(m, 0) for m in range(SWEEP)]
            for i in range(M_TILES):
                plan.append((i, 1))
                if i + SWEEP < M_TILES:
                    plan.append((i + SWEEP, 0))

            acc_of = {}
            ot_of = {}
            for u_idx, (m, n) in enumerate(plan):
                ms = slice(m * P, (m + 1) * P)
                if m not in acc_of:
                    acc_of[m] = {}
                    ot_of[m] = opool.tile([P, U], _F16, name="ot")
                n0, nw = N_CHUNKS[n]
                acc = accp.tile([P, 512], _F32, name="acc", tag="acc")
                acc_of[m][n] = acc
                for k, (k0, kc) in enumerate(K_CHUNKS):
                    nc.tensor.matmul(acc[:, 0:nw],
                                     x_v[0:kc, k, ms],
                                     w_v[0:kc, k, n0:n0 + nw],
                                     start=(k == 0), stop=(k == NK - 1))
                ot = ot_of[m]
                if m == M_TILES - 1:
                    # split the final tile's copies+stores across both HWDGE
                    # rings; the kernel tail is one 360-column store
                    nc.vector.tensor_copy(ot[:, n0:n0 + nw], acc[:, 0:nw])
                    eng = nc.sync if n == 0 else nc.scalar
                    eng.dma_start(out_d[ms, n0:n0 + nw], ot[:, n0:n0 + nw])
                else:
                    nc.vector.tensor_copy(ot[:, n0:n0 + nw], acc[:, 0:nw])
                    if n == 1:
                        # whole-tile store once both halves are cast
                        eng = nc.sync if m >= M_TILES - 3 else nc.scalar
                        eng.dma_start(out_d[ms, :], ot[:])

    nc.compile()
    return nc


def _fold_weights(trend_w, seasonal_w, trend_b, seasonal_b):
    """W = seasonal_w + (trend_w - seasonal_w) @ A via the banded structure of
    A; returns [KE, U] = [W^T; b] ready for the device."""
    trend_w = np.asarray(trend_w, dtype=np.float64)
    seasonal_w = np.asarray(seasonal_w, dtype=np.float64)
    trend_b = np.asarray(trend_b, dtype=np.float64)
    seasonal_b = np.asarray(seasonal_b, dtype=np.float64)
    counts = np.minimum(np.arange(T) + 1, WINDOW).astype(np.float64)
    G = (trend_w - seasonal_w) / counts[None, :]
    M = np.zeros_like(G)
    for d in range(WINDOW):
        M[:, :T - d] += G[:, d:]
    W = seasonal_w + M
    b = trend_b + seasonal_b
    wt_ext = np.empty((KE, U), np.float32)
    wt_ext[:T, :] = W.T.astype(np.float32)
    wt_ext[T, :] = b.astype(np.float32)
    return wt_ext


_NC_CACHE = {}
RUN_KWARGS = {}   # test harness may set {"trace": True}
LAST_RESULTS = None


def kernel(x, trend_w, trend_b, seasonal_w, seasonal_b):
    global LAST_RESULTS
    wt_ext = _fold_weights(trend_w, seasonal_w, trend_b, seasonal_b)
    wt16 = np.zeros((KP, U), np.float16)
    wt16[:KE] = wt_ext.astype(np.float16)

    # Pre-transposed, ones-row-extended, 768-row-padded fp16 shards.
    x2d = np.asarray(x, dtype=np.float32).reshape(M_TOT, T)
    xt_all = np.zeros((KP, M_TOT), np.float16)
    xt_all[:T] = x2d.T.astype(np.float16)
    xt_all[T] = 1.0
    xt_cores = np.ascontiguousarray(
        xt_all.reshape(KP, N_CORES, M_LOC).transpose(1, 0, 2))

    if "nc" not in _NC_CACHE:
        _NC_CACHE["nc"] = _build_nc()
    nc = _NC_CACHE["nc"]

    in_maps = [{"xt": xt_cores[i], "wt": wt16} for i in range(N_CORES)]
    res = run_bass_kernel_spmd(nc, in_maps, core_ids=list(range(N_CORES)),
                               **RUN_KWARGS)
    LAST_RESULTS = res
    out = np.concatenate([r["out"] for r in res.results], axis=0)
    return out.astype(np.float32).reshape(B, NPTS, U)


# revision 6
# speedup vs baseline: 1.2296x; 1.0489x over previous
"""DLinear (causal-window decomposition + dual Linear) as a single fused matmul
on 8 Trainium2 NeuronCores.

Algebra: with A the [T,T] causal-window-mean operator (banded, window=25),
    trend = x @ A^T
    out   = trend @ Tw^T + (x - trend) @ Sw^T + (tb + sb)
          = x @ (Sw + (Tw - Sw) @ A)^T + (tb + sb)
A is banded (25 nonzeros/row), so (Tw-Sw)@A folds on the host in O(T^2) via a
reversed windowed column-sum.  The device then runs one [2048,721] x [721,720]
matmul per core; the bias rides as an extra contraction row against a ones row
appended to x^T.

The per-core x shard is shipped pre-transposed and padded to 768 rows
([768, 2048], contraction dim on partitions) so the device does no PE
transposes and each column-group loads with a single 3D-AP DMA.  Data moves
as fp16 (x, W, out) with fp32 PSUM accumulation: fp16xfp16 products are
exact in fp32, end-to-end error ~5e-4 of output scale while halving DMA
traffic.  Output is upcast to fp32 on the host.

Measured mechanics (NTFF traces): matmul cadence = N cols @ 2.4 GHz with
LDWEIGHTS hidden, so the 16*6*720-column stream is a hard ~28.8us; the input
DMA ramp (~160 GB/s early) sets the ~4.4us first-matmul time; the junk-matmul
warmup must keep the PE busy straight through that ramp or the HAM clock gate
re-throttles the whole stream; and each store costs ~1us of Act-sequencer
time.  The only slack left was the tail, so the last tiles' stores moved to
the by-then-idle sync ring and the final tile stores per 360-column half on
both rings, ending the kernel on a single small transfer."""

import sys
import types

import numpy as np

import concourse.bacc as bacc
import concourse.mybir as mybir
from concourse import tile
from concourse.bass_utils import run_bass_kernel_spmd

# bass_utils imports antenv.axon_hooks when tracing is requested (e.g. a
# BASS_TRACE=1 environment); some images lack that module.  Provide a no-op
# shim so the run degrades to untraced instead of crashing.
try:
    import antenv.axon_hooks  # noqa: F401
except ImportError:
    try:
        import antenv
        _shim = types.ModuleType("antenv.axon_hooks")
        _shim._hook = None
        _shim.set_axon_ntff_profile_hook = lambda h: setattr(_shim, "_hook", h)
        _shim.get_axon_ntff_profile_hook = lambda: _shim._hook
        sys.modules["antenv.axon_hooks"] = _shim
        antenv.axon_hooks = _shim
    except ImportError:
        pass

WINDOW = 25
B, NPTS, T = 32, 512, 720
U = T                     # output features
N_CORES = 8
M_TOT = B * NPTS          # 16384 rows
M_LOC = M_TOT // N_CORES  # 2048 rows per core
P = 128                   # partitions
M_TILES = M_LOC // P      # 16
KE = T + 1                # contraction incl. bias row
K_CHUNKS = [(k * P, min(P, KE - k * P)) for k in range((KE + P - 1) // P)]
NK = len(K_CHUNKS)        # 6
KP = NK * P               # 768, row-padded contraction extent
N_CHUNKS = [(0, 360), (360, 360)]  # PSUM-bank-sized slices of U
# x column-group widths for pipelined loading; first small to start PE early
GROUPS = [(0, 256), (256, 384), (640, 512), (1152, 512), (1664, 384)]

_F32 = mybir.dt.float32
_F16 = mybir.dt.float16
N_WARMUP = 13             # junk matmuls to lift the PE HAM clock-gate
N_FILLER = 1              # junk matmuls after each early unit (DMA-stall gap fill)
FILLER_UNITS = 6          # how many leading plan units get filler
SYNC_STORES = 2           # trailing whole-tile stores moved to the sync ring


def _build_nc():
    nc = bacc.Bacc("TRN2", target_bir_lowering=False, debug=False,
                   num_devices=N_CORES, enable_partition_id=False)
    xt_d = nc.dram_tensor("xt", [KP, M_LOC], _F16, kind="ExternalInput").ap()
    wt_d = nc.dram_tensor("wt", [KP, U], _F16, kind="ExternalInput").ap()
    out_d = nc.dram_tensor("out", [M_LOC, U], _F16, kind="ExternalOutput").ap()
    xt_v = xt_d.rearrange("(k p) m -> p k m", p=P)
    wt_v = wt_d.rearrange("(k p) u -> p k u", p=P)

    with tile.TileContext(nc) as tc:
        with tc.tile_pool(name="wpool", bufs=1) as wpool, \
             tc.tile_pool(name="xpool", bufs=1) as xpool, \
             tc.tile_pool(name="opool", bufs=6) as opool, \
             tc.tile_pool(name="wup", bufs=1, space="PSUM") as wup, \
             tc.tile_pool(name="accp", bufs=7, space="PSUM") as accp:

            # HAM warm-up: junk matmuls keep the PE busy while the first
            # DMAs land, so real matmuls start at the 2.4 GHz clock.
            scr = wpool.tile([P, 384], _F16, name="scr", tag="scr")
            nc.gpsimd.memset(scr[:], 0.0)
            ps_scr = wup.tile([P, 384], _F32, name="ps_scr", tag="ps_scr")
            for _ in range(N_WARMUP):
                nc.tensor.matmul(ps_scr[:], scr[:, 0:P], scr[:],
                                 start=True, stop=True)

            # Inputs all on the HWDGE sync queue, in gate order:
            # w-n0 half, x group 0, w-n1 half, remaining x groups.
            w_all = wpool.tile([P, NK * U], _F16, name="w_all", tag="w_all")
            w_v = w_all[:].rearrange("p (k u) -> p k u", k=NK)
            x_all = xpool.tile([P, NK * M_LOC], _F16, name="x_all", tag="x_all")
            x_v = x_all[:].rearrange("p (k m) -> p k m", k=NK)

            nc.sync.dma_start(w_v[:, :, 0:360], wt_v[:, :, 0:360])
            for c0, cw in GROUPS[:2]:
                nc.sync.dma_start(x_v[:, :, c0:c0 + cw], xt_v[:, :, c0:c0 + cw])
            nc.sync.dma_start(w_v[:, :, 360:720], wt_v[:, :, 360:720])
            for c0, cw in GROUPS[2:]:
                nc.sync.dma_start(x_v[:, :, c0:c0 + cw], xt_v[:, :, c0:c0 + cw])

            # (m, n) schedule: n0-sweep across the m-tiles of groups 0-1 so
            # every later DMA (wn1, g2+) has multi-us slack against
            # completion-latency jitter; m-major afterwards.
            sweep_ms = list(range((GROUPS[1][0] + GROUPS[1][1]) // P))
            plan = [(m, 0) for m in sweep_ms] + [(m, 1) for m in sweep_ms]
            for c0, cw in GROUPS[2:]:
                plan += [(m, n) for m in range(c0 // P, (c0 + cw) // P)
                         for n in (0, 1)]

            ot_tiles = {}
            done = {}
            last_m = plan[-1][0]
            for u_idx, (m, n) in enumerate(plan):
                if m not in ot_tiles:
                    ot_tiles[m] = opool.tile([P, U], _F16, name="ot")
                n0, nw = N_CHUNKS[n]
                acc = accp.tile([P, 512], _F32, name="acc", tag="acc")
                for k, (k0, kc) in enumerate(K_CHUNKS):
                    nc.tensor.matmul(
                        acc[:, 0:nw],
                        x_v[0:kc, k, m * P:(m + 1) * P],
                        w_v[0:kc, k, n0:n0 + nw],
                        start=(k == 0), stop=(k == NK - 1))
                if u_idx < FILLER_UNITS:
                    for _ in range(N_FILLER):
                        nc.tensor.matmul(ps_scr[:], scr[:, 0:P], scr[:],
                                         start=True, stop=True)
                nc.vector.tensor_copy(ot_tiles[m][:, n0:n0 + nw], acc[:, 0:nw])
                done[m] = done.get(m, 0) + 1
                if m == last_m:
                    # the final tile stores per half on both HWDGE rings so
                    # the kernel ends on one 360-column transfer, not a
                    # whole-tile store serialized on the Act sequencer
                    eng = nc.sync if n == 0 else nc.scalar
                    eng.dma_start(out_d[m * P:(m + 1) * P, n0:n0 + nw],
                                  ot_tiles[m][:, n0:n0 + nw])
                elif done[m] == 2:
                    # trailing tiles store via the sync ring, idle after
                    # inputs; earlier tiles use the scalar ring
                    pos = sum(1 for mm in done if done[mm] == 2)
                    eng = (nc.sync if pos > M_TILES - 1 - SYNC_STORES
                           else nc.scalar)
                    eng.dma_start(out_d[m * P:(m + 1) * P, :],
                                  ot_tiles.pop(m)[:])

    nc.compile()
    return nc


def _fold_weights(trend_w, seasonal_w, trend_b, seasonal_b):
    """W = seasonal_w + (trend_w - seasonal_w) @ A via the banded structure of
    A; returns [KE, U] = [W^T; b] ready for the device."""
    trend_w = np.asarray(trend_w, dtype=np.float64)
    seasonal_w = np.asarray(seasonal_w, dtype=np.float64)
    trend_b = np.asarray(trend_b, dtype=np.float64)
    seasonal_b = np.asarray(seasonal_b, dtype=np.float64)
    counts = np.minimum(np.arange(T) + 1, WINDOW).astype(np.float64)
    G = (trend_w.astype(np.float64) - seasonal_w.astype(np.float64)) / counts[None, :]
    M = np.zeros_like(G)
    for d in range(WINDOW):
        M[:, :T - d] += G[:, d:]
    W = seasonal_w.astype(np.float64) + M
    b = trend_b.astype(np.float64) + seasonal_b.astype(np.float64)
    wt_ext = np.empty((KE, U), np.float32)
    wt_ext[:T, :] = W.T.astype(np.float32)
    wt_ext[T, :] = b.astype(np.float32)
    return wt_ext


_NC_CACHE = {}
RUN_KWARGS = {}   # test harness may set {"trace": True}
LAST_RESULTS = None


def kernel(x, trend_w, trend_b, seasonal_w, seasonal_b):
    global LAST_RESULTS
    wt_ext = _fold_weights(trend_w, seasonal_w, trend_b, seasonal_b)

    # Pre-transposed, ones-row-extended, 768-row-padded fp16 shards.
    x2d = np.asarray(x, dtype=np.float32).reshape(M_TOT, T)
    xt_all = np.zeros((KP, M_TOT), np.float16)
    xt_all[:T] = x2d.T.astype(np.float16)
    xt_all[T] = 1.0
    xt_cores = np.ascontiguousarray(
        xt_all.reshape(KP, N_CORES, M_LOC).transpose(1, 0, 2))

    wt16 = np.zeros((KP, U), np.float16)
    wt16[:KE] = wt_ext.astype(np.float16)

    if "nc" not in _NC_CACHE:
        _NC_CACHE["nc"] = _build_nc()
    nc = _NC_CACHE["nc"]

    in_maps = [{"xt": xt_cores[i], "wt": wt16} for i in range(N_CORES)]
    res = run_bass_kernel_spmd(nc, in_maps, core_ids=list(range(N_CORES)),
                               **RUN_KWARGS)
    LAST_RESULTS = res
    out = np.concatenate([r["out"] for r in res.results], axis=0)
    return out.astype(np.float32).reshape(B, NPTS, U)


# revision 7
# speedup vs baseline: 1.2324x; 1.0023x over previous
"""DLinear (causal-window decomposition + dual Linear) as a single fused matmul
on 8 Trainium2 NeuronCores.

Algebra: with A the [T,T] causal-window-mean operator (banded, window=25),
    trend = x @ A^T
    out   = trend @ Tw^T + (x - trend) @ Sw^T + (tb + sb)
          = x @ (Sw + (Tw - Sw) @ A)^T + (tb + sb)
A is banded (25 nonzeros/row), so (Tw-Sw)@A folds on the host in O(T^2) via a
reversed windowed column-sum.  The device then runs one [2048,721] x [721,720]
matmul per core; the bias rides as an extra contraction row against a ones row
appended to x^T.

The per-core x shard is shipped pre-transposed and padded to 768 rows
([768, 2048], contraction dim on partitions) so the device does no PE
transposes and each column-group loads with a single 3D-AP DMA.  Data moves
as fp16 (x, W, out) with fp32 PSUM accumulation: fp16xfp16 products are
exact in fp32, end-to-end error ~5e-4 of output scale while halving DMA
traffic.  Output is upcast to fp32 on the host.

Measured mechanics (NTFF traces): matmul cadence = N cols @ 2.4 GHz with
LDWEIGHTS hidden, so the 16*6*720-column stream is a hard ~28.8us; the input
DMA ramp (~160 GB/s early) sets the ~4.4us first-matmul time; the junk-matmul
warmup must keep the PE busy straight through that ramp or the HAM clock gate
re-throttles the whole stream; and each store costs ~1us of Act-sequencer
time.  The only slack left was the tail, so the last tiles' stores moved to
the by-then-idle sync ring and the final tile stores per 360-column half on
both rings, ending the kernel on a single small transfer."""

import sys
import types

import numpy as np

import concourse.bacc as bacc
import concourse.mybir as mybir
from concourse import tile
from concourse.bass_utils import run_bass_kernel_spmd

# bass_utils imports antenv.axon_hooks when tracing is requested (e.g. a
# BASS_TRACE=1 environment); some images lack that module.  Provide a no-op
# shim so the run degrades to untraced instead of crashing.
try:
    import antenv.axon_hooks  # noqa: F401
except ImportError:
    try:
        import antenv
        _shim = types.ModuleType("antenv.axon_hooks")
        _shim._hook = None
        _shim.set_axon_ntff_profile_hook = lambda h: setattr(_shim, "_hook", h)
        _shim.get_axon_ntff_profile_hook = lambda: _shim._hook
        sys.modules["antenv.axon_hooks"] = _shim
        antenv.axon_hooks = _shim
    except ImportError:
        pass

WINDOW = 25
B, NPTS, T = 32, 512, 720
U = T                     # output features
N_CORES = 8
M_TOT = B * NPTS          # 16384 rows
M_LOC = M_TOT // N_CORES  # 2048 rows per core
P = 128                   # partitions
M_TILES = M_LOC // P      # 16
KE = T + 1                # contraction incl. bias row
K_CHUNKS = [(k * P, min(P, KE - k * P)) for k in range((KE + P - 1) // P)]
NK = len(K_CHUNKS)        # 6
KP = NK * P               # 768, row-padded contraction extent
N_CHUNKS = [(0, 360), (360, 360)]  # PSUM-bank-sized slices of U
# x column-group widths for pipelined loading; first small to start PE early
GROUPS = [(0, 256), (256, 384), (640, 512), (1152, 512), (1664, 384)]

_F32 = mybir.dt.float32
_F16 = mybir.dt.float16
N_WARMUP = 13             # junk matmuls to lift the PE HAM clock-gate
N_FILLER = 1              # junk matmuls after each early unit (DMA-stall gap fill)
FILLER_UNITS = 6          # how many leading plan units get filler
SYNC_STORES = 2           # trailing whole-tile stores moved to the sync ring


def _build_nc():
    nc = bacc.Bacc("TRN2", target_bir_lowering=False, debug=False,
                   num_devices=N_CORES, enable_partition_id=False)
    xt_d = nc.dram_tensor("xt", [KP, M_LOC], _F16, kind="ExternalInput").ap()
    wt_d = nc.dram_tensor("wt", [KP, U], _F16, kind="ExternalInput").ap()
    out_d = nc.dram_tensor("out", [M_LOC, U], _F16, kind="ExternalOutput").ap()
    xt_v = xt_d.rearrange("(k p) m -> p k m", p=P)
    wt_v = wt_d.rearrange("(k p) u -> p k u", p=P)

    with tile.TileContext(nc) as tc:
        with tc.tile_pool(name="wpool", bufs=1) as wpool, \
             tc.tile_pool(name="xpool", bufs=1) as xpool, \
             tc.tile_pool(name="opool", bufs=6) as opool, \
             tc.tile_pool(name="wup", bufs=1, space="PSUM") as wup, \
             tc.tile_pool(name="accp", bufs=7, space="PSUM") as accp:

            # HAM warm-up: junk matmuls keep the PE busy while the first
            # DMAs land, so real matmuls start at the 2.4 GHz clock.
            scr = wpool.tile([P, 384], _F16, name="scr", tag="scr")
            nc.gpsimd.memset(scr[:], 0.0)
            ps_scr = wup.tile([P, 384], _F32, name="ps_scr", tag="ps_scr")
            for _ in range(N_WARMUP):
                nc.tensor.matmul(ps_scr[:], scr[:, 0:P], scr[:],
                                 start=True, stop=True)

            # Inputs all on the HWDGE sync queue, in gate order:
            # w-n0 half, x group 0, w-n1 half, remaining x groups.
            w_all = wpool.tile([P, NK * U], _F16, name="w_all", tag="w_all")
            w_v = w_all[:].rearrange("p (k u) -> p k u", k=NK)
            x_all = xpool.tile([P, NK * M_LOC], _F16, name="x_all", tag="x_all")
            x_v = x_all[:].rearrange("p (k m) -> p k m", k=NK)

            nc.sync.dma_start(w_v[:, :, 0:360], wt_v[:, :, 0:360])
            for c0, cw in GROUPS[:2]:
                nc.sync.dma_start(x_v[:, :, c0:c0 + cw], xt_v[:, :, c0:c0 + cw])
            nc.sync.dma_start(w_v[:, :, 360:720], wt_v[:, :, 360:720])
            for c0, cw in GROUPS[2:]:
                nc.sync.dma_start(x_v[:, :, c0:c0 + cw], xt_v[:, :, c0:c0 + cw])

            # (m, n) schedule: n0-sweep across the m-tiles of groups 0-1 so
            # every later DMA (wn1, g2+) has multi-us slack against
            # completion-latency jitter; m-major afterwards.
            sweep_ms = list(range((GROUPS[1][0] + GROUPS[1][1]) // P))
            plan = [(m, 0) for m in sweep_ms] + [(m, 1) for m in sweep_ms]
            for c0, cw in GROUPS[2:]:
                plan += [(m, n) for m in range(c0 // P, (c0 + cw) // P)
                         for n in (0, 1)]

            ot_tiles = {}
            done = {}
            last_m = plan[-1][0]
            for u_idx, (m, n) in enumerate(plan):
                if m not in ot_tiles:
                    ot_tiles[m] = opool.tile([P, U], _F16, name="ot")
                n0, nw = N_CHUNKS[n]
                if m == last_m and n == 1:
                    # final unit: run as a 256-col + 104-col chain so the
                    # kernel's serial endgame (matmuls -> cast -> store ->
                    # receipt) closes on the small chain (~1us shorter tail);
                    # same total PE cycles, stores split across both rings
                    ml, mh = m * P, (m + 1) * P
                    for c0, cw, eng in ((n0, 256, nc.scalar),
                                        (n0 + 256, nw - 256, nc.sync)):
                        acc = accp.tile([P, 512], _F32, name="acc", tag="acc")
                        for k, (k0, kc) in enumerate(K_CHUNKS):
                            nc.tensor.matmul(
                                acc[:, 0:cw],
                                x_v[0:kc, k, ml:mh],
                                w_v[0:kc, k, c0:c0 + cw],
                                start=(k == 0), stop=(k == NK - 1))
                        nc.vector.tensor_copy(ot_tiles[m][:, c0:c0 + cw],
                                              acc[:, 0:cw])
                        eng.dma_start(out_d[ml:mh, c0:c0 + cw],
                                      ot_tiles[m][:, c0:c0 + cw])
                    continue
                acc = accp.tile([P, 512], _F32, name="acc", tag="acc")
                for k, (k0, kc) in enumerate(K_CHUNKS):
                    nc.tensor.matmul(
                        acc[:, 0:nw],
                        x_v[0:kc, k, m * P:(m + 1) * P],
                        w_v[0:kc, k, n0:n0 + nw],
                        start=(k == 0), stop=(k == NK - 1))
                if u_idx < FILLER_UNITS:
                    for _ in range(N_FILLER):
                        nc.tensor.matmul(ps_scr[:], scr[:, 0:P], scr[:],
                                         start=True, stop=True)
                nc.vector.tensor_copy(ot_tiles[m][:, n0:n0 + nw], acc[:, 0:nw])
                done[m] = done.get(m, 0) + 1
                if m == last_m:
                    # n0 half of the final tile: cast now, store via the
                    # (idle) sync ring; fully hidden under the n1 chains
                    nc.sync.dma_start(out_d[m * P:(m + 1) * P, n0:n0 + nw],
                                      ot_tiles[m][:, n0:n0 + nw])
                elif done[m] == 2:
                    # trailing tiles store via the sync ring, idle after
                    # inputs; earlier tiles use the scalar ring
                    pos = sum(1 for mm in done if done[mm] == 2)
                    eng = (nc.sync if pos > M_TILES - 1 - SYNC_STORES
                           else nc.scalar)
                    eng.dma_start(out_d[m * P:(m + 1) * P, :],
                                  ot_tiles.pop(m)[:])

    nc.compile()
    return nc


def _fold_weights(trend_w, seasonal_w, trend_b, seasonal_b):
    """W = seasonal_w + (trend_w - seasonal_w) @ A via the banded structure of
    A; returns [KE, U] = [W^T; b] ready for the device."""
    trend_w = np.asarray(trend_w, dtype=np.float64)
    seasonal_w = np.asarray(seasonal_w, dtype=np.float64)
    trend_b = np.asarray(trend_b, dtype=np.float64)
    seasonal_b = np.asarray(seasonal_b, dtype=np.float64)
    counts = np.minimum(np.arange(T) + 1, WINDOW).astype(np.float64)
    G = (trend_w.astype(np.float64) - seasonal_w.astype(np.float64)) / counts[None, :]
    M = np.zeros_like(G)
    for d in range(WINDOW):
        M[:, :T - d] += G[:, d:]
    W = seasonal_w.astype(np.float64) + M
    b = trend_b.astype(np.float64) + seasonal_b.astype(np.float64)
    wt_ext = np.empty((KE, U), np.float32)
    wt_ext[:T, :] = W.T.astype(np.float32)
    wt_ext[T, :] = b.astype(np.float32)
    return wt_ext


_NC_CACHE = {}
RUN_KWARGS = {}   # test harness may set {"trace": True}
LAST_RESULTS = None


def kernel(x, trend_w, trend_b, seasonal_w, seasonal_b):
    global LAST_RESULTS
    wt_ext = _fold_weights(trend_w, seasonal_w, trend_b, seasonal_b)

    # Pre-transposed, ones-row-extended, 768-row-padded fp16 shards.
    x2d = np.asarray(x, dtype=np.float32).reshape(M_TOT, T)
    xt_all = np.zeros((KP, M_TOT), np.float16)
    xt_all[:T] = x2d.T.astype(np.float16)
    xt_all[T] = 1.0
    xt_cores = np.ascontiguousarray(
        xt_all.reshape(KP, N_CORES, M_LOC).transpose(1, 0, 2))

    wt16 = np.zeros((KP, U), np.float16)
    wt16[:KE] = wt_ext.astype(np.float16)

    if "nc" not in _NC_CACHE:
        _NC_CACHE["nc"] = _build_nc()
    nc = _NC_CACHE["nc"]

    in_maps = [{"xt": xt_cores[i], "wt": wt16} for i in range(N_CORES)]
    res = run_bass_kernel_spmd(nc, in_maps, core_ids=list(range(N_CORES)),
                               **RUN_KWARGS)
    LAST_RESULTS = res
    out = np.concatenate([r["out"] for r in res.results], axis=0)
    return out.astype(np.float32).reshape(B, NPTS, U)
